# revision 66
# baseline (speedup 1.0000x reference)
"""Trainium2 Bass kernel for block-causal sparse attention (MLA-style KV).

Sharding: tensor-parallel over heads. 16 heads / 8 cores = 2 heads per core,
one KV head per core-pair. Each core computes q/k/v projections from the full
(transposed) x, RoPE, sparse attention for its 2 heads, and a partial output
projection; the host sums the 8 partial outputs.

Sparsity structure (T=4096, BLOCK=128, WINDOW=512, GLOBAL_EVERY=64):
for query block b, visible keys are blocks b-4..b (block b-4 masked by a fixed
triangular+global pattern) plus "global" columns j%64==0 with j < 128*(b-4).

The q/k/v projections run as 3-term fp8 DoubleRow matmuls (x and the
folded weights ship as fp8e4m3 hi/lo pairs, weights pre-scaled by 64; the
hi*hi + hi*lo + lo*hi terms recover ~bf16 accuracy at 2x PE rate over a
256-deep contraction). Everything else runs in bf16 with fp32 PSUM
accumulation; the RoPE half-rotation is a PE permutation matmul.

Scores are computed transposed ([k, q] layout) so probabilities feed the PV
and output-projection matmuls with no transposes. Softmax denominators are
accumulated with ones-matrix matmuls directly in broadcast form ([128, q] in
PSUM), so the reciprocal feeds the normalize multiply without a partition
broadcast; the normalize chain is emission-deferred so its PE ops slot
behind independent work.

The PE stream is kept dense by software-pipelining at emission level: the
q1 projection pass drips into h0's attention items, and the previous tile's
output projection ("wo filler" pairs) fills the remaining stall points
(projection-pass RoPE chains, h1's exp chains). DMAs are consolidated and
laid out so descriptors stay >= 512B (x in 1KB-row chunks, weights
pre-swizzled on the host to the SBUF image layout, full-T rope tables,
row-batched output stores; the last tile stores per-512-column so the drain
pipelines).
"""

import numpy as np

N_CORES = 8
T = 4096
C = 2048
L = 512
H = 16
KVH = 4
HD = 128
BLOCK = 128
WINDOW = 512
GLOBAL_EVERY = 64
ROPE_THETA = 10000.0

QTW = 512            # query tile width (4 blocks)
NQT = T // QTW       # 8
NKT = C // 128       # 16 contraction tiles for projections
NG = T // GLOBAL_EVERY  # 64 global columns

_CACHE = {}


def _build_module():
    import concourse.bacc as bacc
    import concourse.mybir as mybir
    import concourse.tile as tile
    from contextlib import ExitStack

    F32 = mybir.dt.float32
    F32R = mybir.dt.float32r
    BF16 = mybir.dt.bfloat16
    FP8 = mybir.dt.float8e4
    DR = mybir.MatmulPerfMode.DoubleRow
    EXP = mybir.ActivationFunctionType.Exp

    nc = bacc.Bacc("TRN2", target_bir_lowering=False, debug=False,
                   num_devices=N_CORES)

    # x and the projection weights ship as fp8 hi/lo pairs (weights
    # pre-scaled by WS=64 on the host); projections run as 3-term DoubleRow
    # fp8 matmuls (hi*hi + hi*lo + lo*hi), which the PE executes at 2x rate
    # over a 256-deep contraction.
    xh = nc.dram_tensor("xh", [C, T], FP8, kind="ExternalInput")
    xl = nc.dram_tensor("xl", [C, T], FP8, kind="ExternalInput")
    wnames = ["wk", "wq0", "wv", "wq1"]
    wdram = {}
    # weights ship pre-swizzled to the SBUF image layout [128, NKT*HD]
    # so the DMA descriptors are 1KB-contiguous (no small-transfer penalty)
    for wn in wnames:
        wdram[wn + "h"] = nc.dram_tensor(wn + "h", [128, NKT * HD], FP8,
                                         kind="ExternalInput")
        wdram[wn + "l"] = nc.dram_tensor(wn + "l", [128, NKT * HD], FP8,
                                         kind="ExternalInput")
    wo = nc.dram_tensor("wo", [2 * HD, C], BF16, kind="ExternalInput")
    cosd = nc.dram_tensor("cosd", [HD, T], BF16, kind="ExternalInput")
    sind = nc.dram_tensor("sind", [HD, T], BF16, kind="ExternalInput")  # sign-folded
    maskt = nc.dram_tensor("maskt", [128, 128], BF16, kind="ExternalInput")
    maskg = nc.dram_tensor("maskg", [NG, T], BF16, kind="ExternalInput")
    onesd = nc.dram_tensor("onesd", [128, 128], BF16, kind="ExternalInput")
    identd = nc.dram_tensor("identd", [128, 128], BF16, kind="ExternalInput")
    swapd = nc.dram_tensor("swapd", [128, 128], BF16, kind="ExternalInput")
    identfd = nc.dram_tensor("identfd", [128, 128], F32R, kind="ExternalInput")
    out = nc.dram_tensor("out", [T, C], BF16, kind="ExternalOutput")

    WS = 64.0
    scale = 1.0 / np.sqrt(HD) / (WS * WS)

    with tile.TileContext(nc) as tc, ExitStack() as ctx:
        res = ctx.enter_context(tc.tile_pool(name="res", bufs=1))
        kT = res.tile([128, T], BF16, tag="kT")
        vN = res.tile([128, T], BF16, tag="vN")
        kG = res.tile([128, NG], BF16, tag="kG")
        vG = res.tile([64, 128], BF16, tag="vG")
        vGT = res.tile([128, NG], BF16, tag="vGT")
        mT = res.tile([128, 128], BF16, tag="mT")
        mG = res.tile([NG, T], BF16, tag="mG")
        ones = res.tile([128, 128], BF16, tag="ones")
        ident = res.tile([128, 128], BF16, tag="ident")
        swpm = res.tile([128, 128], BF16, tag="swpm")
        identf = res.tile([128, 128], F32R, tag="identf")
        wo_sb = res.tile([128, 2 * C], BF16, tag="wo_sb")
        wsb = {}
        for wn in wnames:
            for sf in ("h", "l"):
                wsb[wn + sf] = res.tile([128, NKT * HD], FP8, tag=wn + sf,
                                        name=wn + sf)
        cosF = res.tile([128, T], BF16, tag="cosF")
        sinF = res.tile([128, T], BF16, tag="sinF")

        xpool = ctx.enter_context(tc.tile_pool(name="xpool", bufs=3))
        qlp = ctx.enter_context(tc.tile_pool(name="qlp", bufs=2))
        vtp = ctx.enter_context(tc.tile_pool(name="vtp", bufs=2))
        swp = ctx.enter_context(tc.tile_pool(name="swp", bufs=2))
        tmpp = ctx.enter_context(tc.tile_pool(name="tmpp", bufs=2))
        ppool = ctx.enter_context(tc.tile_pool(name="ppool", bufs=3))
        ynp = ctx.enter_context(tc.tile_pool(name="ynp", bufs=2))
        recp = ctx.enter_context(tc.tile_pool(name="recp", bufs=2))
        obp = ctx.enter_context(tc.tile_pool(name="obp", bufs=2))
        oblp = ctx.enter_context(tc.tile_pool(name="oblp", bufs=8))

        pjps = ctx.enter_context(tc.tile_pool(name="pjps", bufs=2, space="PSUM"))
        spool = ctx.enter_context(tc.tile_pool(name="spool", bufs=2, space="PSUM"))
        ypool = ctx.enter_context(tc.tile_pool(name="ypool", bufs=1, space="PSUM"))
        dpool = ctx.enter_context(tc.tile_pool(name="dpool", bufs=1, space="PSUM"))
        opool = ctx.enter_context(tc.tile_pool(name="opool", bufs=2, space="PSUM"))

        # ---- deferred output-projection "filler" steps ------------------
        # Each step emits the 2-matmul PSUM pair for one (qs, n) output tile
        # plus its PSUM->SBUF copy and (once a row is complete) the DMA.
        wo_state = {"steps": [], "idx": 0}
        head_norm = {"fn": None}
        iter_norm = {"fn": None}

        def make_wo_steps(ynorm, qs0, last=False):
            steps = []
            obs = {}

            def step(qs, n):
                def run():
                    if not last and n == 0:
                        obs[qs] = obp.tile([128, 2048], BF16, tag="ob", name="ob")
                    o_ps = opool.tile([128, 512], F32, tag="o", name="o_ps")
                    nc.tensor.matmul(o_ps[:], ynorm[0][:, qs * 128:(qs + 1) * 128],
                                     wo_sb[:, n * 512:n * 512 + 512],
                                     start=True, stop=False)
                    nc.tensor.matmul(o_ps[:], ynorm[1][:, qs * 128:(qs + 1) * 128],
                                     wo_sb[:, C + n * 512:C + n * 512 + 512],
                                     start=False, stop=True)
                    rows = slice(qs0 + qs * 128, qs0 + (qs + 1) * 128)
                    if last:
                        # final tile: per-(qs,n) stores so the drain pipelines
                        obl = oblp.tile([128, 512], BF16, tag="obl", name="obl")
                        if (qs * 4 + n) % 2 == 0:
                            nc.scalar.mul(obl[:], o_ps[:], 1.0 / WS)
                        else:
                            nc.vector.tensor_scalar_mul(obl[:], o_ps[:],
                                                        1.0 / WS)
                        nc.sync.dma_start(out[rows, n * 512:(n + 1) * 512],
                                          obl[:])
                        return
                    ob = obs[qs]
                    if (qs * 4 + n) % 2 == 0:
                        nc.scalar.mul(ob[:, n * 512:(n + 1) * 512], o_ps[:],
                                      1.0 / WS)
                    else:
                        nc.vector.tensor_scalar_mul(
                            ob[:, n * 512:(n + 1) * 512], o_ps[:], 1.0 / WS)
                    if n == 3:
                        rows = slice(qs0 + qs * 128, qs0 + (qs + 1) * 128)
                        nc.sync.dma_start(out[rows, :], ob[:])
                return run

            for qs in range(4):
                for n in range(4):
                    steps.append(step(qs, n))
            return steps

        def fill(n):
            st = wo_state
            while n > 0 and st["idx"] < len(st["steps"]):
                st["steps"][st["idx"]]()
                st["idx"] += 1
                n -= 1

        def fill_all():
            fill(len(wo_state["steps"]))

        for it in range(NQT):
            nt = it
            b0 = 4 * it
            ts = slice(nt * 512, (nt + 1) * 512)
            qs0 = it * QTW

            if iter_norm["fn"] is not None:
                iter_norm["fn"]()
                iter_norm["fn"] = None

            # ---- x / weight DMAs (consolidated). At it=0 each pass's
            # weights load in half-tile chunks just ahead of the x chunks
            # that pass will consume, so the PE starts ~2.5us in. ----
            def emit_w_half(wn, half):
                hw = NKT * HD // 2
                for sf in ("h", "l"):
                    nc.sync.dma_start(
                        wsb[wn + sf][:, half * hw:(half + 1) * hw],
                        wdram[wn + sf][:, half * hw:(half + 1) * hw])

            if it == 0:
                emit_w_half("wk", 0)
                nc.gpsimd.dma_start(ident[:], identd[:])
                nc.gpsimd.dma_start(mT[:], maskt[:])
                nc.gpsimd.dma_start(ones[:], onesd[:])
                nc.gpsimd.dma_start(swpm[:], swapd[:])
                nc.gpsimd.dma_start(identf[:], identfd[:])
            xh_sb = xpool.tile([128, NKT * 512], FP8, tag="xh")
            xl_sb = xpool.tile([128, NKT * 512], FP8, tag="xl")
            w_after = {0: [("wk", 1)], 1: [("wq0", 0)], 2: [("wq0", 1)],
                       3: [("wv", 0)], 4: [("wv", 1)], 5: ["cs0"],
                       6: [("wq1", 0)], 7: [("wq1", 1), "csrest"]} \
                if it == 0 else {}
            for q8 in range(8):
                for xsb, xdr in ((xh_sb, xh), (xl_sb, xl)):
                    nc.sync.dma_start(
                        xsb[:, q8 * 1024:(q8 + 1) * 1024].rearrange(
                            "p (a t) -> p a t", a=2),
                        xdr[q8 * 256:(q8 + 1) * 256, ts].rearrange(
                            "(a p) t -> p a t", p=128))
                for wspec in w_after.get(q8, ()):
                    if wspec == "cs0":
                        # just the slice tile 0's RoPE needs, so it lands early
                        nc.sync.dma_start(cosF[:, 0:512], cosd[:, 0:512])
                        nc.sync.dma_start(sinF[:, 0:512], sind[:, 0:512])
                    elif wspec == "csrest":
                        nc.sync.dma_start(cosF[:, 512:], cosd[:, 512:])
                        nc.sync.dma_start(sinF[:, 512:], sind[:, 512:])
                    else:
                        emit_w_half(*wspec)

            cos_t = cosF[:, ts]
            sin_t = sinF[:, ts]

            qloc = [qlp.tile([128, 512], BF16, tag=f"ql{h}", name=f"ql{h}")
                    for h in range(2)]
            # pass order k, q0, v, q1: each RoPE chain hides under the
            # following projection passes so kT/qloc are ready for attention
            vT_t = vtp.tile([128, 512], BF16, tag="vT")
            ropedest = [kT[:, ts], qloc[0][:], None, qloc[1][:]]

            def emit_pass_cp(i, pj, cp):
                wn = wnames[i]
                wh = wsb[wn + "h"][:, 2 * cp * 128:(2 * cp + 2) * 128]\
                    .rearrange("p (a m) -> p a m", a=2)
                wl = wsb[wn + "l"][:, 2 * cp * 128:(2 * cp + 2) * 128]\
                    .rearrange("p (a m) -> p a m", a=2)
                xhp = xh_sb[:, 2 * cp * 512:(2 * cp + 2) * 512]\
                    .rearrange("p (a t) -> p a t", a=2)
                xlp = xl_sb[:, 2 * cp * 512:(2 * cp + 2) * 512]\
                    .rearrange("p (a t) -> p a t", a=2)
                nc.tensor.matmul(pj[:], wh, xhp, perf_mode=DR,
                                 start=(cp == 0), stop=False)
                nc.tensor.matmul(pj[:], wl, xhp, perf_mode=DR,
                                 start=False, stop=False)
                nc.tensor.matmul(pj[:], wh, xlp, perf_mode=DR,
                                 start=False, stop=(cp == NKT // 2 - 1))

            def emit_pass_tail(i, pj):
                if i != 2:
                    # RoPE: dest = qsb*cos + swap(qsb)*sinS; the half-rotation
                    # runs on the PE (permutation matmul) to keep it off the
                    # DMA queues
                    dest = ropedest[i]
                    qsb = swp.tile([128, 512], BF16, tag="qsb")
                    nc.scalar.copy(qsb[:], pj[:])
                    sw_ps = spool.tile([128, QTW], F32, tag="s", name="sw_ps")
                    nc.tensor.matmul(sw_ps[:], swpm[:], qsb[:],
                                     start=True, stop=True)
                    ta = tmpp.tile([128, 512], BF16, tag="ta")
                    nc.vector.tensor_mul(ta[:], qsb[:], cos_t)
                    tb = tmpp.tile([128, 512], BF16, tag="tb")
                    nc.vector.tensor_mul(tb[:], sw_ps[:], sin_t)
                    nc.vector.tensor_add(dest, ta[:], tb[:])
                else:
                    nc.vector.tensor_copy(vT_t[:], pj[:])

            # passes k, q0, v run eagerly; pass q1 is deferred and dripped
            # into h0's attention items as PE filler
            for i in range(3):
                pj = pjps.tile([128, 512], F32, tag="pj")
                for cp in range(NKT // 2):
                    emit_pass_cp(i, pj, cp)
                emit_pass_tail(i, pj)
                fill(2)
            pj_q1 = pjps.tile([128, 512], F32, tag="pj")
            q1_state = {"cp": 0}

            def drip_q1(n):
                while n > 0 and q1_state["cp"] < NKT // 2:
                    emit_pass_cp(3, pj_q1, q1_state["cp"])
                    q1_state["cp"] += 1
                    if q1_state["cp"] == NKT // 2:
                        emit_pass_tail(3, pj_q1)
                    n -= 1

            if it == 0:
                nc.gpsimd.dma_start(mG[:], maskg[:])
                for i in range(2):
                    nc.sync.dma_start(wo_sb[:, i * C:(i + 1) * C],
                                      wo[i * 128:(i + 1) * 128, :])

            # ---- v transpose for this t-tile + incremental global K/V ----
            fill(4)
            for j in range(4):
                blk = nt * 4 + j
                tp = spool.tile([128, 512], BF16, tag="s", name="tp")
                nc.tensor.transpose(tp[:, :128], vT_t[:, j * 128:(j + 1) * 128],
                                    ident[:])
                nc.vector.tensor_copy(vN[:, blk * 128:(blk + 1) * 128], tp[:, :128])
            gsl = slice(nt * 8, (nt + 1) * 8)
            nc.vector.tensor_copy(kG[:, gsl], kT[:, ts][:, 0:512:GLOBAL_EVERY])
            nc.vector.tensor_copy(vGT[:, gsl], vT_t[:][:, 0:512:GLOBAL_EVERY])
            gw2 = 8 * (nt + 1)
            tpg = spool.tile([128, 512], BF16, tag="s", name="tpg")
            nc.tensor.transpose(tpg[:gw2, :128], vGT[:, :gw2], ident[:])
            nc.vector.tensor_copy(vG[:gw2, :], tpg[:gw2, :128])

            # ---- attention for query tile `it` (4 blocks b0..b0+3) ----
            gw = min(NG, 8 * it)   # written prefix of kG/vG; 0 for it=0
            ynorm = []
            for h in range(2):
                items = [(b0, 0, 512, None)]
                if it == 0:
                    for j in range(3):
                        items.append((j + 1, (j + 1) * 128, (3 - j) * 128, None))
                    use_glob = False
                else:
                    for j in range(4):
                        items.append((b0 - 4 + j, 0, (j + 1) * 128, j))
                    for j in range(3):
                        items.append((b0 + 1 + j, (j + 1) * 128, (3 - j) * 128, None))
                    use_glob = gw > 0

                y_ps = ypool.tile([128, QTW], F32, tag="y")
                d_ps = dpool.tile([128, QTW], F32, tag="d")
                n_items = len(items) + (1 if use_glob else 0)
                s_tiles = [None] * n_items

                def emit_qk(ii):
                    s = spool.tile([128, QTW], F32, tag="s")
                    if ii < len(items):
                        kb, qoff, w, _ = items[ii]
                        nc.tensor.matmul(
                            s[:, :w], kT[:, kb * 128:(kb + 1) * 128],
                            qloc[h][:, qoff:qoff + w],
                            start=True, stop=True)
                    else:
                        nc.tensor.matmul(s[:gw, :], kG[:, :gw], qloc[h][:],
                                         start=True, stop=True)
                    s_tiles[ii] = s

                def emit_rest(ii):
                    first = ii == 0
                    last = ii == n_items - 1
                    s = s_tiles[ii]
                    p = ppool.tile([128, QTW], BF16, tag="p")
                    if ii < len(items):
                        kb, qoff, w, tri = items[ii]
                        nc.scalar.activation(p[:, :w], s[:, :w], EXP, scale=scale)
                        if tri is not None:
                            nc.vector.tensor_mul(p[:, tri * 128:(tri + 1) * 128],
                                                 p[:, tri * 128:(tri + 1) * 128],
                                                 mT[:])
                        nc.tensor.matmul(y_ps[:, qoff:qoff + w],
                                         vN[:, kb * 128:(kb + 1) * 128], p[:, :w],
                                         start=first, stop=last)
                        nc.tensor.matmul(d_ps[:, qoff:qoff + w], ones[:, :],
                                         p[:, :w], start=first, stop=last)
                    else:
                        nc.scalar.activation(p[:gw, :], s[:gw, :], EXP, scale=scale)
                        nc.vector.tensor_mul(p[:gw, :], p[:gw, :],
                                             mG[:gw, qs0:qs0 + QTW])
                        nc.tensor.matmul(y_ps[:, :], vG[:gw, :], p[:gw, :],
                                         start=first, stop=last)
                        nc.tensor.matmul(d_ps[:, :], ones[:gw, :], p[:gw, :],
                                         start=first, stop=last)

                emit_qk(0)
                for ii in range(n_items):
                    if ii + 1 < n_items:
                        emit_qk(ii + 1)
                    if h == 1 and ii == 0 and head_norm["fn"] is not None:
                        head_norm["fn"]()
                        head_norm["fn"] = None
                    if h == 0:
                        drip_q1(2)
                    else:
                        fill(1)
                    emit_rest(ii)
                if h == 0:
                    drip_q1(NKT // 2)

                # d2 [q, 4] -> reciprocal -> transpose -> broadcast
                # matmul so the per-q reciprocal lands replicated across
                # partitions ([128, q] in PSUM) for the normalize multiply.
                # Deferred so the PE-side ops slot behind other PE work.
                yn = ynp.tile([128, QTW], BF16, tag=f"yn{h}", name=f"yn{h}")
                ynorm.append(yn)

                def make_norm(y_ps=y_ps, d_ps=d_ps, yn=yn):
                    def go():
                        rbc = recp.tile([128, QTW], F32, tag="rbc")
                        nc.vector.reciprocal(rbc[:], d_ps[:])
                        nc.vector.tensor_mul(yn[:], y_ps[:], rbc[:])
                    return go

                if h == 0:
                    head_norm["fn"] = make_norm()
                else:
                    iter_norm["fn"] = make_norm()
                fill(2)

            # ---- output projection: deferred as filler for the next
            # iteration's stall points ----
            fill_all()
            wo_state = {"steps": make_wo_steps(ynorm, qs0, last=(it == NQT - 1)),
                        "idx": 0}

        if iter_norm["fn"] is not None:
            iter_norm["fn"]()
            iter_norm["fn"] = None
        fill_all()

    nc.compile()
    return nc


def _host_inputs(x, w_q, w_kv_down, w_k_up, w_v_up, w_o):
    """Build the per-core input maps (host-side shard + precompute)."""
    import ml_dtypes
    BF = ml_dtypes.bfloat16
    E4 = ml_dtypes.float8_e4m3  # TRN2's F8E4M3 (not the -fn variant)
    WS = 64.0
    x = np.asarray(x)
    w_q = np.asarray(w_q)
    w_kv_down = np.asarray(w_kv_down)
    w_k_up = np.asarray(w_k_up)
    w_v_up = np.asarray(w_v_up)
    w_o = np.asarray(w_o)
    x2 = np.ascontiguousarray(x.reshape(T, C).astype(np.float32))
    xt = np.ascontiguousarray(x2.T)

    def hilo(a):
        hi = a.astype(E4)
        lo = (a - hi.astype(np.float32)).astype(E4)
        return np.ascontiguousarray(hi), np.ascontiguousarray(lo)

    def hilo_img(a):
        # [C, HD] -> SBUF image [128, NKT*HD]
        img = a.reshape(NKT, 128, HD).transpose(1, 0, 2).reshape(128, NKT * HD)
        return hilo(img)

    xt_h, xt_l = hilo(xt)

    # RoPE tables, [hd, t] layout, sign folded into sin for the swapped term
    freqs = 1.0 / (ROPE_THETA ** (np.arange(0, HD, 2, dtype=np.float64) / HD))
    emb = np.arange(T, dtype=np.float64)[:, None] * freqs[None, :]   # [T, 64]
    cos = np.concatenate([np.cos(emb), np.cos(emb)], axis=-1)        # [T, 128]
    sin = np.concatenate([np.sin(emb), np.sin(emb)], axis=-1)
    cosT = np.ascontiguousarray(cos.T.astype(BF))                    # [128, T]
    sinS = sin.T.copy()
    sinS[:64, :] *= -1.0
    sinS = np.ascontiguousarray(sinS.astype(BF))

    # fixed triangular+global mask for the b-4 key block, [k_off, q_off]
    oi = np.arange(128)
    mTm = ((oi[None, :] <= oi[:, None]) | (oi[:, None] % 64 == 0)).astype(BF)

    # global-column mask [g, q]: visible iff 64 g < 128 (q//128 - 4)
    g = np.arange(NG)
    qb = np.arange(T) // BLOCK
    mGm = (64 * g[:, None] < 128 * (qb[None, :] - 4)).astype(BF)

    onesm = np.ones((128, 128), BF)
    ident = np.eye(128, dtype=BF)
    # swap matrix: out[m] = in[(m+64)%128]  (matmul form: swapm[k,m]=1 iff
    # k == (m+64)%128)
    km = np.arange(128)
    swapm = (km[:, None] == (km[None, :] + 64) % 128).astype(BF)
    identf32 = np.eye(128, dtype=np.float32)

    wk_f = (w_kv_down.astype(np.float32) @ w_k_up.astype(np.float32))  # [C, KVH*HD]
    wv_f = (w_kv_down.astype(np.float32) @ w_v_up.astype(np.float32))

    in_maps = []
    for c in range(N_CORES):
        h0 = 2 * c
        kv = h0 // (H // KVH)
        wq0_h, wq0_l = hilo_img(w_q[:, h0 * HD:(h0 + 1) * HD].astype(np.float32) * WS)
        wq1_h, wq1_l = hilo_img(w_q[:, (h0 + 1) * HD:(h0 + 2) * HD].astype(np.float32) * WS)
        wk_h, wk_l = hilo_img(wk_f[:, kv * HD:(kv + 1) * HD] * WS)
        wv_h, wv_l = hilo_img(wv_f[:, kv * HD:(kv + 1) * HD] * WS)
        wo_c = np.ascontiguousarray(
            w_o[h0 * HD:(h0 + 2) * HD, :].astype(BF))
        in_maps.append({
            "xh": xt_h, "xl": xt_l,
            "wq0h": wq0_h, "wq0l": wq0_l, "wq1h": wq1_h, "wq1l": wq1_l,
            "wkh": wk_h, "wkl": wk_l, "wvh": wv_h, "wvl": wv_l,
            "wo": wo_c,
            "cosd": cosT, "sind": sinS, "maskt": mTm, "maskg": mGm,
            "onesd": onesm, "identd": ident, "swapd": swapm,
            "identfd": identf32,
        })
    return in_maps


def _get_module():
    if "nc" not in _CACHE:
        _CACHE["nc"] = _build_module()
    return _CACHE["nc"]


def kernel(x, w_q, w_kv_down, w_k_up, w_v_up, w_o):
    from concourse.bass_utils import run_bass_kernel_spmd

    nc = _get_module()
    in_maps = _host_inputs(x, w_q, w_kv_down, w_k_up, w_v_up, w_o)
    res = run_bass_kernel_spmd(nc, in_maps, list(range(N_CORES)))
    acc = np.zeros((T, C), np.float32)
    for c in range(N_CORES):
        acc += np.asarray(res.results[c]["out"], dtype=np.float32)
    return acc.reshape(1, T, C)
